# revision 16
# baseline (speedup 1.0000x reference)
"""Chamfer distance on 8 Trainium2 NeuronCores.

Problem: x1 (8, 4096, 3) f32, y1 (8, 4096, 3) f32.
  d2[b,m,n] = |y[b,m] - x[b,n]|^2
  out = mean_{b,n}(min_m sqrt(1e-8 + max(d2,0))) + mean_{b,m}(min_n ...)

Strategy (data-parallel over B, one batch element per core):
  * sqrt / +eps / max(.,0) are monotonic -> compute mins over raw d2 and
    apply them only to the reduced 4096-vectors.
  * -d2 = -(y_sq + x_sq - 2 x.y) is produced directly in PSUM by a
    single matmul with augmented K=30 inputs: each fp32 operand is
    split into 3 bf16 levels (~24-bit effective mantissa) so the result
    sits at the reference's own fp32 noise floor, while the bf16 matmul
    streams at 1 cycle/row (fp32 is 4x slower). The y side is negated
    so on-device mins become maxes (enables the DVE MAX8 unit).
  * each PSUM tile has exactly ONE consumer — the scalar engine casts it
    to bf16 in SBUF (multi-engine PSUM consumers are serialized by the
    tile scheduler's bank tracker and would gate the PE). All reduction
    work then runs on the DVE at bf16 rates: direction "min1" (over m)
    as an elementwise running max at the 2x_1P rate, direction "min2"
    (over n) as a pair-max halving TT plus a MAX8 top-8 scan.
  * epilogue: the min1 accumulator is PE-transposed so the partition
    direction becomes the free axis, then reduced; clamp + sqrt(d2+eps)
    with free-axis sum accumulation on the scalar engine; partition-sum
    on gpsimd. Each core emits [sum_min2, sum_min1]; the host sums
    across cores and divides by B*N.
"""

import os
import sys

for _p in ("/opt/trn_rl_repo", "/root/.axon_site/_ro/trn_rl_repo"):
    if os.path.isdir(_p) and _p not in sys.path:
        sys.path.insert(0, _p)
        break

import numpy as np
import ml_dtypes

_B = 8
_N = 4096          # points per cloud (both x and y)
_K = 30            # augmented contraction dim (3-level bf16 split)
_NCORES = 8
_MT = _N // 128    # 32 m-tiles (partition dim of d2 tiles)
_NCH = 2           # n is processed in 2 chunks of 2048 (4 PSUM banks each)
_CHUNK = _N // _NCH

_BF16 = ml_dtypes.bfloat16

_PROGRAM = None


def _build_program():
    import concourse.bacc as bacc
    import concourse.tile as tile
    import concourse.mybir as mybir
    from concourse.masks import make_identity
    from concourse import bass_isa

    f32 = mybir.dt.float32
    bf16 = mybir.dt.bfloat16
    MAX = mybir.AluOpType.max
    X = mybir.AxisListType.X

    nc = bacc.Bacc("TRN2", target_bir_lowering=False, debug=False,
                   num_devices=_NCORES)

    xh_d = nc.dram_tensor("xh", [_K, _N], bf16, kind="ExternalInput")
    yh_d = nc.dram_tensor("yh", [_K, _N], bf16, kind="ExternalInput")
    out_d = nc.dram_tensor("out", [1, 2], f32, kind="ExternalOutput")

    with tile.TileContext(nc) as tc:
        with tc.tile_pool(name="singles", bufs=1) as singles:
            xh_s = singles.tile([_K, _N], bf16)
            yh_s = singles.tile([_K, _N], bf16)
            nc.sync.dma_start(out=xh_s[:, :], in_=xh_d.ap())
            nc.sync.dma_start(out=yh_s[:, :], in_=yh_d.ap())

            # The PE produces NEGATED d2 (host negates the yh rows), so all
            # mins become maxes — letting direction A use the MAX8 unit.
            # Running max over m (partition direction), bf16, on DVE.
            accBb = singles.tile([128, _N], bf16)
            nc.gpsimd.memset(accBb[:, :], -1.0e30)

            # per-mt MAX8 results (direction A): 8 values per mt, col 0 is
            # the max
            m8all = singles.tile([128, _MT * 8], bf16)

            # Each PSUM tile has exactly ONE consumer (the scalar engine's
            # bf16 cast) — multiple consumers of a PSUM tile are serialized
            # by the tile scheduler's bank tracker, which would gate the PE.
            # All max work then runs off the bf16 SBUF copy.
            with tc.tile_pool(name="psum", bufs=2, space="PSUM") as psum, \
                 tc.tile_pool(name="castp", bufs=4) as castp, \
                 tc.tile_pool(name="halfp", bufs=4) as halfp:
                for mt in range(_MT):
                    lhsT = yh_s[:, mt * 128:(mt + 1) * 128]
                    ptb = castp.tile([128, _N], bf16, name="ptb")
                    for c in range(_NCH):
                        pt = psum.tile([128, _CHUNK], f32)
                        for j in range(_CHUNK // 512):
                            n0 = c * _CHUNK + j * 512
                            nc.tensor.matmul(
                                pt[:, j * 512:(j + 1) * 512],
                                lhsT=lhsT,
                                rhs=xh_s[:, n0:n0 + 512],
                                start=True, stop=True,
                            )
                        nc.scalar.copy(
                            out=ptb[:, c * _CHUNK:(c + 1) * _CHUNK],
                            in_=pt[:, :])
                    # direction B: elementwise running max (bf16 2x mode)
                    nc.vector.tensor_tensor(
                        out=accBb[:, :], in0=accBb[:, :], in1=ptb[:, :],
                        op=MAX)
                    # direction A: pair-max halving, then the MAX8 unit
                    h = halfp.tile([128, _CHUNK], bf16, name="h")
                    nc.vector.tensor_tensor(
                        out=h[:, :], in0=ptb[:, 0:_CHUNK],
                        in1=ptb[:, _CHUNK:_N], op=MAX)
                    nc.vector.max(m8all[:, mt * 8:(mt + 1) * 8], h[:, :])

            # ---- epilogue ----
            dirA = singles.tile([128, _MT], f32)
            nc.vector.tensor_reduce(
                out=dirA[:, :],
                in_=m8all[:, :].rearrange("p (m e) -> p m e", e=8),
                axis=X, op=MAX)

            identb = singles.tile([128, 128], bf16)
            make_identity(nc, identb[:, :])

            dirB = singles.tile([128, _MT], f32)
            with tc.tile_pool(name="tpsum", bufs=4, space="PSUM") as tpsum:
                for b in range(_N // 128):
                    tp = tpsum.tile([128, 128], bf16, name="tpb")
                    nc.tensor.transpose(
                        tp[:, :], accBb[:, b * 128:(b + 1) * 128],
                        identb[:, :])
                    nc.vector.tensor_reduce(
                        out=dirB[:, b:b + 1], in_=tp[:, :],
                        axis=X, op=MAX)

            # dirA/dirB hold M = max(-d2) = -min(d2); the reference computes
            # sqrt(eps + max(d2min, 0)) = sqrt(eps - min(M, 0)).
            sums = singles.tile([128, 2], f32)
            eps_t = singles.tile([128, 1], f32)
            nc.vector.memset(eps_t[:, :], 1.0e-8)
            scratch = singles.tile([128, _MT], f32)
            sqrt_out = singles.tile([128, _MT], f32)
            for col, t in ((0, dirA), (1, dirB)):
                nc.vector.tensor_scalar_min(scratch[:, :], t[:, :], 0.0)
                nc.scalar.activation(
                    out=sqrt_out[:, :], in_=scratch[:, :],
                    func=mybir.ActivationFunctionType.Sqrt,
                    bias=eps_t[:, :], scale=-1.0,
                    accum_out=sums[:, col:col + 1])

            red = singles.tile([128, 2], f32)
            nc.gpsimd.partition_all_reduce(
                red[:, :], sums[:, :], channels=128,
                reduce_op=bass_isa.ReduceOp.add)
            nc.sync.dma_start(out=out_d.ap(), in_=red[0:1, :])

    nc.compile()
    return nc


def _get_program():
    global _PROGRAM
    if _PROGRAM is None:
        _PROGRAM = _build_program()
    return _PROGRAM


def _split3(a):
    """fp32 array -> 3-level bf16 split (h1 + h2 + h3 ~ a to ~2^-26 rel)."""
    h1 = a.astype(_BF16)
    r1 = a - h1.astype(np.float32)
    h2 = r1.astype(_BF16)
    r2 = r1 - h2.astype(np.float32)
    h3 = r2.astype(_BF16)
    return h1, h2, h3


def _augment(x, y):
    """x, y: (4096, 3) f32 -> xh, yh (30, 4096) bf16 such that
    sum_k yh[k, m] * xh[k, n] == |y[m] - x[n]|^2 to ~1e-6 abs.

    Every fp32 operand is split into 3 bf16 levels; all product pairs down
    to the 2^-24 level are kept, so each product is exact in the PE's fp32
    PSUM accumulation.  Large-magnitude rows (y_sq, x_sq, hi*hi cross
    terms) come first so the running PSUM partial cancels down to ~d2
    early, keeping sequential-accumulation rounding at the fp32 noise
    floor of the reference itself."""
    xt = np.ascontiguousarray(x.T.astype(np.float32))            # (3, N)
    y2t = np.ascontiguousarray((-2.0 * y).T.astype(np.float32))  # (3, N)
    xsq = np.einsum("nd,nd->n", x, x).astype(np.float32)         # (N,)
    ysq = np.einsum("nd,nd->n", y, y).astype(np.float32)

    g1, g2, g3 = _split3(xt)
    h1, h2, h3 = _split3(y2t)
    xs1, xs2, xs3 = _split3(xsq)
    ys1, ys2, ys3 = _split3(ysq)
    ones = np.ones(_N, dtype=_BF16)

    xrows, yrows = [], []

    def add(xr, yr):
        xrows.append(xr)
        yrows.append(yr)

    add(ones, ys1)
    add(xs1, ones)
    for d in range(3):
        add(g1[d], h1[d])
    add(ones, ys2)
    add(ones, ys3)
    add(xs2, ones)
    add(xs3, ones)
    for d in range(3):
        add(g2[d], h1[d])
        add(g1[d], h2[d])
        add(g3[d], h1[d])
        add(g2[d], h2[d])
        add(g1[d], h3[d])
        add(g3[d], h2[d])
        add(g2[d], h3[d])
    xh = np.stack(xrows).astype(_BF16)
    # negate the y side so the PE emits -d2 (mins become maxes on-device)
    yh = (-np.stack(yrows).astype(np.float32)).astype(_BF16)
    assert xh.shape == (_K, _N)
    return xh, yh


def kernel(x1, y1):
    from concourse.bass_utils import run_bass_kernel_spmd

    x1 = np.asarray(x1)
    y1 = np.asarray(y1)
    assert x1.shape == (_B, _N, 3) and y1.shape == (_B, _N, 3)

    nc = _get_program()
    in_maps = []
    for b in range(_B):
        xh, yh = _augment(x1[b], y1[b])
        in_maps.append({"xh": xh, "yh": yh})

    res = run_bass_kernel_spmd(nc, in_maps, list(range(_NCORES)))
    total = 0.0
    for c in range(_NCORES):
        o = res.results[c]["out"]
        total += float(o[0, 0]) + float(o[0, 1])
    return np.float32(total / (_B * _N))


# revision 17
# speedup vs baseline: 1.1150x; 1.1150x over previous
"""Chamfer distance on 8 Trainium2 NeuronCores.

Problem: x1 (8, 4096, 3) f32, y1 (8, 4096, 3) f32.
  d2[b,m,n] = |y[b,m] - x[b,n]|^2
  out = mean_{b,n}(min_m sqrt(1e-8 + max(d2,0))) + mean_{b,m}(min_n ...)

Strategy (data-parallel over B, one batch element per core):
  * sqrt / +eps / max(.,0) are monotonic -> compute mins over raw d2 and
    apply them only to the reduced 4096-vectors.
  * -d2 = -(y_sq + x_sq - 2 x.y) is produced directly in PSUM by a
    single matmul with augmented K=30 inputs: each fp32 operand is
    split into 3 bf16 levels (~24-bit effective mantissa) so the result
    sits at the reference's own fp32 noise floor, while the bf16 matmul
    streams at 1 cycle/row (fp32 is 4x slower). The y side is negated
    so on-device mins become maxes (enables the DVE MAX8 unit).
  * each PSUM tile has exactly ONE consumer — the scalar engine casts it
    to bf16 in SBUF (multi-engine PSUM consumers are serialized by the
    tile scheduler's bank tracker and would gate the PE). All reduction
    work then runs on the DVE at bf16 rates: direction "min1" (over m)
    as an elementwise running max at the 2x_1P rate, direction "min2"
    (over n) as a pair-max halving TT plus a MAX8 top-8 scan.
  * epilogue: the min1 accumulator is PE-transposed so the partition
    direction becomes the free axis, then reduced; clamp + sqrt(d2+eps)
    with free-axis sum accumulation on the scalar engine; partition-sum
    on gpsimd. Each core emits [sum_min2, sum_min1]; the host sums
    across cores and divides by B*N.
"""

import os
import sys

for _p in ("/opt/trn_rl_repo", "/root/.axon_site/_ro/trn_rl_repo"):
    if os.path.isdir(_p) and _p not in sys.path:
        sys.path.insert(0, _p)
        break

import numpy as np
import ml_dtypes

_B = 8
_N = 4096          # points per cloud (both x and y)
_K = 30            # augmented contraction dim (3-level bf16 split)
_NCORES = 8
_MT = _N // 128    # 32 m-tiles (partition dim of d2 tiles)
_NCH = 2           # n is processed in 2 chunks of 2048 (4 PSUM banks each)
_CHUNK = _N // _NCH

_BF16 = ml_dtypes.bfloat16

_PROGRAM = None


def _build_program():
    import concourse.bacc as bacc
    import concourse.tile as tile
    import concourse.mybir as mybir
    from concourse.masks import make_identity
    from concourse import bass_isa

    f32 = mybir.dt.float32
    bf16 = mybir.dt.bfloat16
    MAX = mybir.AluOpType.max
    X = mybir.AxisListType.X

    nc = bacc.Bacc("TRN2", target_bir_lowering=False, debug=False,
                   num_devices=_NCORES)

    xh_d = nc.dram_tensor("xh", [_K, _N], bf16, kind="ExternalInput")
    yh_d = nc.dram_tensor("yh", [_K, _N], bf16, kind="ExternalInput")
    out_d = nc.dram_tensor("out", [1, 2], f32, kind="ExternalOutput")

    with tile.TileContext(nc) as tc:
        with tc.tile_pool(name="singles", bufs=1) as singles:
            xh_s = singles.tile([_K, _N], bf16)
            yh_s = singles.tile([_K, _N], bf16)
            nc.sync.dma_start(out=xh_s[:, :], in_=xh_d.ap())
            nc.sync.dma_start(out=yh_s[:, :], in_=yh_d.ap())

            # The PE produces NEGATED d2 (host negates the yh rows), so all
            # mins become maxes — letting direction A use the MAX8 unit.
            # Running max over m (partition direction), bf16, on DVE.
            accBb = singles.tile([128, _N], bf16)
            nc.gpsimd.memset(accBb[:, :], -1.0e30)

            # per-mt MAX8 results (direction A): 8 values per mt, col 0 is
            # the max
            m8all = singles.tile([128, _MT * 8], bf16)

            # Each PSUM tile has exactly ONE consumer (the scalar engine's
            # bf16 cast) — multiple consumers of a PSUM tile are serialized
            # by the tile scheduler's bank tracker, which would gate the PE.
            # All max work then runs off the bf16 SBUF copy.
            with tc.tile_pool(name="psum", bufs=2, space="PSUM") as psum, \
                 tc.tile_pool(name="castp", bufs=4) as castp, \
                 tc.tile_pool(name="halfp", bufs=4) as halfp:
                for mt in range(_MT):
                    lhsT = yh_s[:, mt * 128:(mt + 1) * 128]
                    ptb = castp.tile([128, _N], bf16, name="ptb")
                    for c in range(_NCH):
                        pt = psum.tile([128, _CHUNK], f32)
                        for j in range(_CHUNK // 512):
                            n0 = c * _CHUNK + j * 512
                            nc.tensor.matmul(
                                pt[:, j * 512:(j + 1) * 512],
                                lhsT=lhsT,
                                rhs=xh_s[:, n0:n0 + 512],
                                start=True, stop=True,
                            )
                        nc.scalar.copy(
                            out=ptb[:, c * _CHUNK:(c + 1) * _CHUNK],
                            in_=pt[:, :])
                    # direction B: elementwise running max (bf16 2x mode)
                    nc.vector.tensor_tensor(
                        out=accBb[:, :], in0=accBb[:, :], in1=ptb[:, :],
                        op=MAX)
                    # direction A: 3 pair-max halving levels (2x mode),
                    # then the MAX8 unit on the remaining 512 columns
                    h1 = halfp.tile([128, 2048], bf16, name="h1")
                    nc.vector.tensor_tensor(
                        out=h1[:, :], in0=ptb[:, 0:2048],
                        in1=ptb[:, 2048:4096], op=MAX)
                    h2 = halfp.tile([128, 1024], bf16, name="h2")
                    nc.vector.tensor_tensor(
                        out=h2[:, :], in0=h1[:, 0:1024],
                        in1=h1[:, 1024:2048], op=MAX)
                    h3 = halfp.tile([128, 512], bf16, name="h3")
                    nc.vector.tensor_tensor(
                        out=h3[:, :], in0=h2[:, 0:512],
                        in1=h2[:, 512:1024], op=MAX)
                    nc.vector.max(m8all[:, mt * 8:(mt + 1) * 8], h3[:, :])

            # ---- epilogue ----
            dirA = singles.tile([128, _MT], f32)
            nc.vector.tensor_reduce(
                out=dirA[:, :],
                in_=m8all[:, :].rearrange("p (m e) -> p m e", e=8),
                axis=X, op=MAX)

            identb = singles.tile([128, 128], bf16)
            make_identity(nc, identb[:, :])

            dirB = singles.tile([128, _MT], f32)
            with tc.tile_pool(name="tpsum", bufs=4, space="PSUM") as tpsum:
                for b in range(_N // 128):
                    tp = tpsum.tile([128, 128], bf16, name="tpb")
                    nc.tensor.transpose(
                        tp[:, :], accBb[:, b * 128:(b + 1) * 128],
                        identb[:, :])
                    nc.vector.tensor_reduce(
                        out=dirB[:, b:b + 1], in_=tp[:, :],
                        axis=X, op=MAX)

            # dirA/dirB hold M = max(-d2) = -min(d2); the reference computes
            # sqrt(eps + max(d2min, 0)) = sqrt(eps - min(M, 0)).
            sums = singles.tile([128, 2], f32)
            eps_t = singles.tile([128, 1], f32)
            nc.vector.memset(eps_t[:, :], 1.0e-8)
            scratch = singles.tile([128, _MT], f32)
            sqrt_out = singles.tile([128, _MT], f32)
            for col, t in ((0, dirA), (1, dirB)):
                nc.vector.tensor_scalar_min(scratch[:, :], t[:, :], 0.0)
                nc.scalar.activation(
                    out=sqrt_out[:, :], in_=scratch[:, :],
                    func=mybir.ActivationFunctionType.Sqrt,
                    bias=eps_t[:, :], scale=-1.0,
                    accum_out=sums[:, col:col + 1])

            red = singles.tile([128, 2], f32)
            nc.gpsimd.partition_all_reduce(
                red[:, :], sums[:, :], channels=128,
                reduce_op=bass_isa.ReduceOp.add)
            nc.sync.dma_start(out=out_d.ap(), in_=red[0:1, :])

    nc.compile()
    return nc


def _get_program():
    global _PROGRAM
    if _PROGRAM is None:
        _PROGRAM = _build_program()
    return _PROGRAM


def _split3(a):
    """fp32 array -> 3-level bf16 split (h1 + h2 + h3 ~ a to ~2^-26 rel)."""
    h1 = a.astype(_BF16)
    r1 = a - h1.astype(np.float32)
    h2 = r1.astype(_BF16)
    r2 = r1 - h2.astype(np.float32)
    h3 = r2.astype(_BF16)
    return h1, h2, h3


def _augment(x, y):
    """x, y: (4096, 3) f32 -> xh, yh (30, 4096) bf16 such that
    sum_k yh[k, m] * xh[k, n] == |y[m] - x[n]|^2 to ~1e-6 abs.

    Every fp32 operand is split into 3 bf16 levels; all product pairs down
    to the 2^-24 level are kept, so each product is exact in the PE's fp32
    PSUM accumulation.  Large-magnitude rows (y_sq, x_sq, hi*hi cross
    terms) come first so the running PSUM partial cancels down to ~d2
    early, keeping sequential-accumulation rounding at the fp32 noise
    floor of the reference itself."""
    xt = np.ascontiguousarray(x.T.astype(np.float32))            # (3, N)
    y2t = np.ascontiguousarray((-2.0 * y).T.astype(np.float32))  # (3, N)
    xsq = np.einsum("nd,nd->n", x, x).astype(np.float32)         # (N,)
    ysq = np.einsum("nd,nd->n", y, y).astype(np.float32)

    g1, g2, g3 = _split3(xt)
    h1, h2, h3 = _split3(y2t)
    xs1, xs2, xs3 = _split3(xsq)
    ys1, ys2, ys3 = _split3(ysq)
    ones = np.ones(_N, dtype=_BF16)

    xrows, yrows = [], []

    def add(xr, yr):
        xrows.append(xr)
        yrows.append(yr)

    add(ones, ys1)
    add(xs1, ones)
    for d in range(3):
        add(g1[d], h1[d])
    add(ones, ys2)
    add(ones, ys3)
    add(xs2, ones)
    add(xs3, ones)
    for d in range(3):
        add(g2[d], h1[d])
        add(g1[d], h2[d])
        add(g3[d], h1[d])
        add(g2[d], h2[d])
        add(g1[d], h3[d])
        add(g3[d], h2[d])
        add(g2[d], h3[d])
    xh = np.stack(xrows).astype(_BF16)
    # negate the y side so the PE emits -d2 (mins become maxes on-device)
    yh = (-np.stack(yrows).astype(np.float32)).astype(_BF16)
    assert xh.shape == (_K, _N)
    return xh, yh


def kernel(x1, y1):
    from concourse.bass_utils import run_bass_kernel_spmd

    x1 = np.asarray(x1)
    y1 = np.asarray(y1)
    assert x1.shape == (_B, _N, 3) and y1.shape == (_B, _N, 3)

    nc = _get_program()
    in_maps = []
    for b in range(_B):
        xh, yh = _augment(x1[b], y1[b])
        in_maps.append({"xh": xh, "yh": yh})

    res = run_bass_kernel_spmd(nc, in_maps, list(range(_NCORES)))
    total = 0.0
    for c in range(_NCORES):
        o = res.results[c]["out"]
        total += float(o[0, 0]) + float(o[0, 1])
    return np.float32(total / (_B * _N))


# revision 21
# speedup vs baseline: 1.1272x; 1.0109x over previous
"""Chamfer distance on 8 Trainium2 NeuronCores.

Problem: x1 (8, 4096, 3) f32, y1 (8, 4096, 3) f32.
  d2[b,m,n] = |y[b,m] - x[b,n]|^2
  out = mean_{b,n}(min_m sqrt(1e-8 + max(d2,0))) + mean_{b,m}(min_n ...)

Strategy (data-parallel over B, one batch element per core):
  * sqrt / +eps / max(.,0) are monotonic -> compute mins over raw d2 and
    apply them only to the reduced 4096-vectors.
  * -d2 = -(y_sq + x_sq - 2 x.y) is produced directly in PSUM by a
    single matmul with augmented K=30 inputs: each fp32 operand is
    split into 3 bf16 levels (~24-bit effective mantissa) so the result
    sits at the reference's own fp32 noise floor, while the bf16 matmul
    streams at 1 cycle/row (fp32 is 4x slower). The y side is negated
    so on-device mins become maxes (enables the DVE MAX8 unit).
  * each PSUM tile has exactly ONE consumer — the scalar engine casts it
    to bf16 in SBUF (multi-engine PSUM consumers are serialized by the
    tile scheduler's bank tracker and would gate the PE). All reduction
    work then runs on the DVE at bf16 rates: direction "min1" (over m)
    as an elementwise running max at the 2x_1P rate, direction "min2"
    (over n) as a pair-max halving TT plus a MAX8 top-8 scan.
  * epilogue: the min1 accumulator is PE-transposed so the partition
    direction becomes the free axis, then reduced; clamp + sqrt(d2+eps)
    with free-axis sum accumulation on the scalar engine; partition-sum
    on gpsimd. Each core emits [sum_min2, sum_min1]; the host sums
    across cores and divides by B*N.
"""

import os
import sys

for _p in ("/opt/trn_rl_repo", "/root/.axon_site/_ro/trn_rl_repo"):
    if os.path.isdir(_p) and _p not in sys.path:
        sys.path.insert(0, _p)
        break

import numpy as np
import ml_dtypes

_B = 8
_N = 4096          # points per cloud (both x and y)
_K = 30            # augmented contraction dim (3-level bf16 split)
_NCORES = 8
_MT = _N // 128    # 32 m-tiles (partition dim of d2 tiles)
_NCH = 2           # n is processed in 2 chunks of 2048 (4 PSUM banks each)
_CHUNK = _N // _NCH

_BF16 = ml_dtypes.bfloat16

_PROGRAM = None


def _build_program():
    import concourse.bacc as bacc
    import concourse.tile as tile
    import concourse.mybir as mybir
    from concourse.masks import make_identity
    from concourse import bass_isa

    f32 = mybir.dt.float32
    bf16 = mybir.dt.bfloat16
    MAX = mybir.AluOpType.max
    X = mybir.AxisListType.X

    nc = bacc.Bacc("TRN2", target_bir_lowering=False, debug=False,
                   num_devices=_NCORES)

    xh_d = nc.dram_tensor("xh", [_K, _N], bf16, kind="ExternalInput")
    yh_d = nc.dram_tensor("yh", [_K, _N], bf16, kind="ExternalInput")
    out_d = nc.dram_tensor("out", [1, 2], f32, kind="ExternalOutput")

    with tile.TileContext(nc) as tc:
        with tc.tile_pool(name="singles", bufs=1) as singles:
            xh_s = singles.tile([_K, _N], bf16)
            yh_s = singles.tile([_K, _N], bf16)
            nc.sync.dma_start(out=xh_s[:, :], in_=xh_d.ap())
            nc.sync.dma_start(out=yh_s[:, :], in_=yh_d.ap())

            # The PE produces NEGATED d2 (host negates the yh rows), so all
            # mins become maxes — letting direction A use the MAX8 unit.
            # Running max over m (partition direction), bf16, on DVE.
            # mt 0 initializes it with 4x-rate copies instead of TT+memset.
            accBb = singles.tile([128, _N], bf16)

            # per-mt MAX8 results (direction A): 8 values per mt, col 0 is
            # the max
            m8all = singles.tile([128, _MT * 8], bf16)

            # Each PSUM tile has exactly ONE consumer (the scalar engine's
            # bf16 cast) — multiple consumers of a PSUM tile are serialized
            # by the tile scheduler's bank tracker, which would gate the PE.
            # All max work then runs off the bf16 SBUF copy.
            with tc.tile_pool(name="psum", bufs=2, space="PSUM") as psum, \
                 tc.tile_pool(name="castp", bufs=4) as castp, \
                 tc.tile_pool(name="halfp", bufs=4) as halfp:
                for mt in range(_MT):
                    lhsT = yh_s[:, mt * 128:(mt + 1) * 128]
                    ptb = castp.tile([128, _N], bf16, name="ptb")
                    for c in range(_NCH):
                        pt = psum.tile([128, _CHUNK], f32)
                        for j in range(_CHUNK // 512):
                            n0 = c * _CHUNK + j * 512
                            nc.tensor.matmul(
                                pt[:, j * 512:(j + 1) * 512],
                                lhsT=lhsT,
                                rhs=xh_s[:, n0:n0 + 512],
                                start=True, stop=True,
                            )
                        nc.scalar.copy(
                            out=ptb[:, c * _CHUNK:(c + 1) * _CHUNK],
                            in_=pt[:, :])
                        if mt == 0:
                            nc.vector.tensor_copy(
                                accBb[:, c * _CHUNK:(c + 1) * _CHUNK],
                                ptb[:, c * _CHUNK:(c + 1) * _CHUNK])
                    # direction B: elementwise running max (bf16 2x mode)
                    if mt > 0:
                        nc.vector.tensor_tensor(
                            out=accBb[:, :], in0=accBb[:, :],
                            in1=ptb[:, :], op=MAX)
                    # direction A: 3 pair-max halving levels (2x mode),
                    # then the MAX8 unit on the remaining 512 columns
                    h1 = halfp.tile([128, 2048], bf16, name="h1")
                    nc.vector.tensor_tensor(
                        out=h1[:, :], in0=ptb[:, 0:2048],
                        in1=ptb[:, 2048:4096], op=MAX)
                    h2 = halfp.tile([128, 1024], bf16, name="h2")
                    nc.vector.tensor_tensor(
                        out=h2[:, :], in0=h1[:, 0:1024],
                        in1=h1[:, 1024:2048], op=MAX)
                    h3 = halfp.tile([128, 512], bf16, name="h3")
                    nc.vector.tensor_tensor(
                        out=h3[:, :], in0=h2[:, 0:512],
                        in1=h2[:, 512:1024], op=MAX)
                    nc.vector.max(m8all[:, mt * 8:(mt + 1) * 8], h3[:, :])

            # ---- epilogue ----
            dirA = singles.tile([128, _MT], f32)
            nc.vector.tensor_reduce(
                out=dirA[:, :],
                in_=m8all[:, :].rearrange("p (m e) -> p m e", e=8),
                axis=X, op=MAX)

            identb = singles.tile([128, 128], bf16)
            make_identity(nc, identb[:, :])

            dirB = singles.tile([128, _MT], f32)
            with tc.tile_pool(name="tpsum", bufs=4, space="PSUM") as tpsum:
                for g in range(_N // 512):
                    tp = tpsum.tile([128, 512], bf16, name="tpb")
                    for i in range(4):
                        b = g * 4 + i
                        nc.tensor.transpose(
                            tp[:, i * 128:(i + 1) * 128],
                            accBb[:, b * 128:(b + 1) * 128],
                            identb[:, :])
                    # reduce only the innermost 128 (old partition axis);
                    # the 4 transposed blocks stay separate columns
                    nc.vector.tensor_reduce(
                        out=dirB[:, g * 4:(g + 1) * 4],
                        in_=tp[:, :].rearrange("p (a b) -> p a b", b=128),
                        axis=X, op=MAX)

            # dirA/dirB hold M = max(-d2) = -min(d2); the reference computes
            # sqrt(eps + max(d2min, 0)) = sqrt(eps - min(M, 0)).
            sums = singles.tile([128, 2], f32)
            eps_t = singles.tile([128, 1], f32)
            nc.vector.memset(eps_t[:, :], 1.0e-8)
            scratch = singles.tile([128, _MT], f32)
            sqrt_out = singles.tile([128, _MT], f32)
            for col, t in ((0, dirA), (1, dirB)):
                nc.vector.tensor_scalar_min(scratch[:, :], t[:, :], 0.0)
                nc.scalar.activation(
                    out=sqrt_out[:, :], in_=scratch[:, :],
                    func=mybir.ActivationFunctionType.Sqrt,
                    bias=eps_t[:, :], scale=-1.0,
                    accum_out=sums[:, col:col + 1])

            red = singles.tile([128, 2], f32)
            nc.gpsimd.partition_all_reduce(
                red[:, :], sums[:, :], channels=128,
                reduce_op=bass_isa.ReduceOp.add)
            nc.sync.dma_start(out=out_d.ap(), in_=red[0:1, :])

    nc.compile()
    return nc


def _get_program():
    global _PROGRAM
    if _PROGRAM is None:
        _PROGRAM = _build_program()
    return _PROGRAM


def _split3(a):
    """fp32 array -> 3-level bf16 split (h1 + h2 + h3 ~ a to ~2^-26 rel)."""
    h1 = a.astype(_BF16)
    r1 = a - h1.astype(np.float32)
    h2 = r1.astype(_BF16)
    r2 = r1 - h2.astype(np.float32)
    h3 = r2.astype(_BF16)
    return h1, h2, h3


def _augment(x, y):
    """x, y: (4096, 3) f32 -> xh, yh (30, 4096) bf16 such that
    sum_k yh[k, m] * xh[k, n] == |y[m] - x[n]|^2 to ~1e-6 abs.

    Every fp32 operand is split into 3 bf16 levels; all product pairs down
    to the 2^-24 level are kept, so each product is exact in the PE's fp32
    PSUM accumulation.  Large-magnitude rows (y_sq, x_sq, hi*hi cross
    terms) come first so the running PSUM partial cancels down to ~d2
    early, keeping sequential-accumulation rounding at the fp32 noise
    floor of the reference itself."""
    xt = np.ascontiguousarray(x.T.astype(np.float32))            # (3, N)
    y2t = np.ascontiguousarray((-2.0 * y).T.astype(np.float32))  # (3, N)
    xsq = np.einsum("nd,nd->n", x, x).astype(np.float32)         # (N,)
    ysq = np.einsum("nd,nd->n", y, y).astype(np.float32)

    g1, g2, g3 = _split3(xt)
    h1, h2, h3 = _split3(y2t)
    xs1, xs2, xs3 = _split3(xsq)
    ys1, ys2, ys3 = _split3(ysq)
    ones = np.ones(_N, dtype=_BF16)

    xrows, yrows = [], []

    def add(xr, yr):
        xrows.append(xr)
        yrows.append(yr)

    add(ones, ys1)
    add(xs1, ones)
    for d in range(3):
        add(g1[d], h1[d])
    add(ones, ys2)
    add(ones, ys3)
    add(xs2, ones)
    add(xs3, ones)
    for d in range(3):
        add(g2[d], h1[d])
        add(g1[d], h2[d])
        add(g3[d], h1[d])
        add(g2[d], h2[d])
        add(g1[d], h3[d])
        add(g3[d], h2[d])
        add(g2[d], h3[d])
    xh = np.stack(xrows).astype(_BF16)
    # negate the y side so the PE emits -d2 (mins become maxes on-device)
    yh = (-np.stack(yrows).astype(np.float32)).astype(_BF16)
    assert xh.shape == (_K, _N)
    return xh, yh


def kernel(x1, y1):
    from concourse.bass_utils import run_bass_kernel_spmd

    x1 = np.asarray(x1)
    y1 = np.asarray(y1)
    assert x1.shape == (_B, _N, 3) and y1.shape == (_B, _N, 3)

    nc = _get_program()
    in_maps = []
    for b in range(_B):
        xh, yh = _augment(x1[b], y1[b])
        in_maps.append({"xh": xh, "yh": yh})

    res = run_bass_kernel_spmd(nc, in_maps, list(range(_NCORES)))
    total = 0.0
    for c in range(_NCORES):
        o = res.results[c]["out"]
        total += float(o[0, 0]) + float(o[0, 1])
    return np.float32(total / (_B * _N))


# revision 26
# speedup vs baseline: 1.1377x; 1.0093x over previous
"""Chamfer distance on 8 Trainium2 NeuronCores.

Problem: x1 (8, 4096, 3) f32, y1 (8, 4096, 3) f32.
  d2[b,m,n] = |y[b,m] - x[b,n]|^2
  out = mean_{b,n}(min_m sqrt(1e-8 + max(d2,0))) + mean_{b,m}(min_n ...)

Strategy (data-parallel over B, one batch element per core):
  * sqrt / +eps / max(.,0) are monotonic -> compute mins over raw d2 and
    apply them only to the reduced 4096-vectors.
  * -d2 = -(y_sq + x_sq - 2 x.y) is produced directly in PSUM by a
    single matmul with augmented K=30 inputs: each fp32 operand is
    split into 3 bf16 levels (~24-bit effective mantissa) so the result
    sits at the reference's own fp32 noise floor, while the bf16 matmul
    streams at 1 cycle/row (fp32 is 4x slower). The y side is negated
    so on-device mins become maxes (enables the DVE MAX8 unit).
  * each PSUM tile has exactly ONE consumer — the scalar engine casts it
    to bf16 in SBUF (multi-engine PSUM consumers are serialized by the
    tile scheduler's bank tracker and would gate the PE). All reduction
    work then runs on the DVE at bf16 rates: direction "min1" (over m)
    as an elementwise running max at the 2x_1P rate, direction "min2"
    (over n) as a pair-max halving TT plus a MAX8 top-8 scan.
  * epilogue: the min1 accumulator is PE-transposed so the partition
    direction becomes the free axis, then reduced; clamp + sqrt(d2+eps)
    with free-axis sum accumulation on the scalar engine; partition-sum
    on gpsimd. Each core emits [sum_min2, sum_min1]; the host sums
    across cores and divides by B*N.
"""

import os
import sys

for _p in ("/opt/trn_rl_repo", "/root/.axon_site/_ro/trn_rl_repo"):
    if os.path.isdir(_p) and _p not in sys.path:
        sys.path.insert(0, _p)
        break

import numpy as np
import ml_dtypes

_B = 8
_N = 4096          # points per cloud (both x and y)
_K = 30            # augmented contraction dim (3-level bf16 split)
_NCORES = 8
_MT = _N // 128    # 32 m-tiles (partition dim of d2 tiles)
_NCH = 2           # n is processed in 2 chunks of 2048 (4 PSUM banks each)
_CHUNK = _N // _NCH

_BF16 = ml_dtypes.bfloat16

_PROGRAM = None


def _build_program():
    import concourse.bacc as bacc
    import concourse.tile as tile
    import concourse.mybir as mybir
    from concourse.masks import make_identity
    from concourse import bass_isa

    f32 = mybir.dt.float32
    bf16 = mybir.dt.bfloat16
    MAX = mybir.AluOpType.max
    X = mybir.AxisListType.X

    nc = bacc.Bacc("TRN2", target_bir_lowering=False, debug=False,
                   num_devices=_NCORES)

    xh_d = nc.dram_tensor("xh", [_K, _N], bf16, kind="ExternalInput")
    yh_d = nc.dram_tensor("yh", [_K, _N], bf16, kind="ExternalInput")
    out_d = nc.dram_tensor("out", [128, 2 * _MT], f32,
                           kind="ExternalOutput")

    with tile.TileContext(nc) as tc:
        with tc.tile_pool(name="singles", bufs=1) as singles:
            xh_s = singles.tile([_K, _N], bf16)
            yh_s = singles.tile([_K, _N], bf16)
            nc.sync.dma_start(out=xh_s[:, :], in_=xh_d.ap())
            nc.sync.dma_start(out=yh_s[:, :], in_=yh_d.ap())

            # The PE produces NEGATED d2 (host negates the yh rows), so all
            # mins become maxes — letting direction A use the MAX8 unit.
            # Running max over m (partition direction), bf16, on DVE.
            # mt 0 initializes it with 4x-rate copies instead of TT+memset.
            accBb = singles.tile([128, _N], bf16)

            # per-mt MAX8 results (direction A): 8 values per mt, col 0 is
            # the max
            m8all = singles.tile([128, _MT * 8], bf16)

            # Each PSUM tile has exactly ONE consumer (the scalar engine's
            # bf16 cast) — multiple consumers of a PSUM tile are serialized
            # by the tile scheduler's bank tracker, which would gate the PE.
            # All max work then runs off the bf16 SBUF copy.
            with tc.tile_pool(name="psum", bufs=2, space="PSUM") as psum, \
                 tc.tile_pool(name="castp", bufs=4) as castp, \
                 tc.tile_pool(name="halfp", bufs=4) as halfp:
                for mt in range(_MT):
                    lhsT = yh_s[:, mt * 128:(mt + 1) * 128]
                    ptb = castp.tile([128, _N], bf16, name="ptb")
                    for c in range(_NCH):
                        pt = psum.tile([128, _CHUNK], f32)
                        for j in range(_CHUNK // 512):
                            n0 = c * _CHUNK + j * 512
                            nc.tensor.matmul(
                                pt[:, j * 512:(j + 1) * 512],
                                lhsT=lhsT,
                                rhs=xh_s[:, n0:n0 + 512],
                                start=True, stop=True,
                            )
                        if mt == 0:
                            # DVE is idle during the ramp: cast mt0 itself
                            # (psum stays single-consumer; ACT starts at mt1)
                            nc.vector.tensor_copy(
                                ptb[:, c * _CHUNK:(c + 1) * _CHUNK],
                                pt[:, :])
                            nc.vector.tensor_copy(
                                accBb[:, c * _CHUNK:(c + 1) * _CHUNK],
                                ptb[:, c * _CHUNK:(c + 1) * _CHUNK])
                        else:
                            nc.scalar.copy(
                                out=ptb[:, c * _CHUNK:(c + 1) * _CHUNK],
                                in_=pt[:, :])
                    # direction B: elementwise running max (bf16 2x mode)
                    if mt > 0:
                        nc.vector.tensor_tensor(
                            out=accBb[:, :], in0=accBb[:, :],
                            in1=ptb[:, :], op=MAX)
                    # direction A: 3 pair-max halving levels (2x mode),
                    # then the MAX8 unit on the remaining 512 columns
                    h1 = halfp.tile([128, 2048], bf16, name="h1")
                    nc.vector.tensor_tensor(
                        out=h1[:, :], in0=ptb[:, 0:2048],
                        in1=ptb[:, 2048:4096], op=MAX)
                    h2 = halfp.tile([128, 1024], bf16, name="h2")
                    nc.vector.tensor_tensor(
                        out=h2[:, :], in0=h1[:, 0:1024],
                        in1=h1[:, 1024:2048], op=MAX)
                    h3 = halfp.tile([128, 512], bf16, name="h3")
                    nc.vector.tensor_tensor(
                        out=h3[:, :], in0=h2[:, 0:512],
                        in1=h2[:, 512:1024], op=MAX)
                    nc.vector.max(m8all[:, mt * 8:(mt + 1) * 8], h3[:, :])

            # ---- epilogue ----
            dirA = singles.tile([128, _MT], f32)
            nc.vector.tensor_reduce(
                out=dirA[:, :],
                in_=m8all[:, :].rearrange("p (m e) -> p m e", e=8),
                axis=X, op=MAX)

            identb = singles.tile([128, 128], bf16)
            make_identity(nc, identb[:, :])

            dirB = singles.tile([128, _MT], f32)
            with tc.tile_pool(name="tpsum", bufs=4, space="PSUM") as tpsum:
                for g in range(_N // 512):
                    tp = tpsum.tile([128, 512], bf16, name="tpb")
                    for i in range(4):
                        b = g * 4 + i
                        nc.tensor.transpose(
                            tp[:, i * 128:(i + 1) * 128],
                            accBb[:, b * 128:(b + 1) * 128],
                            identb[:, :])
                    # reduce only the innermost 128 (old partition axis);
                    # the 4 transposed blocks stay separate columns
                    nc.vector.tensor_reduce(
                        out=dirB[:, g * 4:(g + 1) * 4],
                        in_=tp[:, :].rearrange("p (a b) -> p a b", b=128),
                        axis=X, op=MAX)

            # dirA/dirB hold M = max(-d2) = -min(d2) for 2*4096 points; the
            # final clamp/sqrt/sum runs on the host (0.4% of the FLOPs),
            # which avoids the Sqrt act-table load and the serial tail.
            nc.sync.dma_start(out=out_d.ap()[:, 0:_MT], in_=dirA[:, :])
            nc.sync.dma_start(out=out_d.ap()[:, _MT:2 * _MT], in_=dirB[:, :])

    nc.compile()
    return nc


def _get_program():
    global _PROGRAM
    if _PROGRAM is None:
        _PROGRAM = _build_program()
    return _PROGRAM


def _split3(a):
    """fp32 array -> 3-level bf16 split (h1 + h2 + h3 ~ a to ~2^-26 rel)."""
    h1 = a.astype(_BF16)
    r1 = a - h1.astype(np.float32)
    h2 = r1.astype(_BF16)
    r2 = r1 - h2.astype(np.float32)
    h3 = r2.astype(_BF16)
    return h1, h2, h3


def _augment(x, y):
    """x, y: (4096, 3) f32 -> xh, yh (30, 4096) bf16 such that
    sum_k yh[k, m] * xh[k, n] == |y[m] - x[n]|^2 to ~1e-6 abs.

    Every fp32 operand is split into 3 bf16 levels; all product pairs down
    to the 2^-24 level are kept, so each product is exact in the PE's fp32
    PSUM accumulation.  Large-magnitude rows (y_sq, x_sq, hi*hi cross
    terms) come first so the running PSUM partial cancels down to ~d2
    early, keeping sequential-accumulation rounding at the fp32 noise
    floor of the reference itself."""
    xt = np.ascontiguousarray(x.T.astype(np.float32))            # (3, N)
    y2t = np.ascontiguousarray((-2.0 * y).T.astype(np.float32))  # (3, N)
    xsq = np.einsum("nd,nd->n", x, x).astype(np.float32)         # (N,)
    ysq = np.einsum("nd,nd->n", y, y).astype(np.float32)

    g1, g2, g3 = _split3(xt)
    h1, h2, h3 = _split3(y2t)
    xs1, xs2, xs3 = _split3(xsq)
    ys1, ys2, ys3 = _split3(ysq)
    ones = np.ones(_N, dtype=_BF16)

    xrows, yrows = [], []

    def add(xr, yr):
        xrows.append(xr)
        yrows.append(yr)

    add(ones, ys1)
    add(xs1, ones)
    for d in range(3):
        add(g1[d], h1[d])
    add(ones, ys2)
    add(ones, ys3)
    add(xs2, ones)
    add(xs3, ones)
    for d in range(3):
        add(g2[d], h1[d])
        add(g1[d], h2[d])
        add(g3[d], h1[d])
        add(g2[d], h2[d])
        add(g1[d], h3[d])
        add(g3[d], h2[d])
        add(g2[d], h3[d])
    xh = np.stack(xrows).astype(_BF16)
    # negate the y side so the PE emits -d2 (mins become maxes on-device)
    yh = (-np.stack(yrows).astype(np.float32)).astype(_BF16)
    assert xh.shape == (_K, _N)
    return xh, yh


def kernel(x1, y1):
    from concourse.bass_utils import run_bass_kernel_spmd

    x1 = np.asarray(x1)
    y1 = np.asarray(y1)
    assert x1.shape == (_B, _N, 3) and y1.shape == (_B, _N, 3)

    nc = _get_program()
    in_maps = []
    for b in range(_B):
        xh, yh = _augment(x1[b], y1[b])
        in_maps.append({"xh": xh, "yh": yh})

    res = run_bass_kernel_spmd(nc, in_maps, list(range(_NCORES)))
    total = 0.0
    for c in range(_NCORES):
        m = res.results[c]["out"].astype(np.float32)  # (128, 64) = -d2min
        dist = np.sqrt(1.0e-8 + np.maximum(-m, 0.0), dtype=np.float32)
        total += float(dist.sum(dtype=np.float64))
    return np.float32(total / (_B * _N))


# revision 28
# speedup vs baseline: 1.1402x; 1.0022x over previous
"""Chamfer distance on 8 Trainium2 NeuronCores.

Problem: x1 (8, 4096, 3) f32, y1 (8, 4096, 3) f32.
  d2[b,m,n] = |y[b,m] - x[b,n]|^2
  out = mean_{b,n}(min_m sqrt(1e-8 + max(d2,0))) + mean_{b,m}(min_n ...)

Strategy (data-parallel over B, one batch element per core):
  * sqrt / +eps / max(.,0) are monotonic -> compute mins over raw d2 and
    apply them only to the reduced 4096-vectors.
  * -d2 = -(y_sq + x_sq - 2 x.y) is produced directly in PSUM by a
    single matmul with augmented K=30 inputs: each fp32 operand is
    split into 3 bf16 levels (~24-bit effective mantissa) so the result
    sits at the reference's own fp32 noise floor, while the bf16 matmul
    streams at 1 cycle/row (fp32 is 4x slower). The y side is negated
    so on-device mins become maxes (enables the DVE MAX8 unit).
  * each PSUM tile has exactly ONE consumer — the scalar engine casts it
    to bf16 in SBUF (multi-engine PSUM consumers are serialized by the
    tile scheduler's bank tracker and would gate the PE). All reduction
    work then runs on the DVE at bf16 rates: direction "min1" (over m)
    as an elementwise running max at the 2x_1P rate, direction "min2"
    (over n) as a pair-max halving TT plus a MAX8 top-8 scan.
  * epilogue: the min1 accumulator is PE-transposed so the partition
    direction becomes the free axis, then reduced; clamp + sqrt(d2+eps)
    with free-axis sum accumulation on the scalar engine; partition-sum
    on gpsimd. Each core emits [sum_min2, sum_min1]; the host sums
    across cores and divides by B*N.
"""

import os
import sys

for _p in ("/opt/trn_rl_repo", "/root/.axon_site/_ro/trn_rl_repo"):
    if os.path.isdir(_p) and _p not in sys.path:
        sys.path.insert(0, _p)
        break

import numpy as np
import ml_dtypes

_B = 8
_N = 4096          # points per cloud (both x and y)
_K = 30            # augmented contraction dim (3-level bf16 split)
_NCORES = 8
_MT = _N // 128    # 32 m-tiles (partition dim of d2 tiles)
_NCH = 2           # n is processed in 2 chunks of 2048 (4 PSUM banks each)
_CHUNK = _N // _NCH

_BF16 = ml_dtypes.bfloat16

_PROGRAM = None


def _build_program():
    import concourse.bacc as bacc
    import concourse.tile as tile
    import concourse.mybir as mybir
    from concourse.masks import make_identity
    from concourse import bass_isa

    f32 = mybir.dt.float32
    bf16 = mybir.dt.bfloat16
    MAX = mybir.AluOpType.max
    X = mybir.AxisListType.X

    nc = bacc.Bacc("TRN2", target_bir_lowering=False, debug=False,
                   num_devices=_NCORES)

    xh_d = nc.dram_tensor("xh", [_K, _N], bf16, kind="ExternalInput")
    yh_d = nc.dram_tensor("yh", [_K, _N], bf16, kind="ExternalInput")
    out_d = nc.dram_tensor("out", [128, 2 * _MT], f32,
                           kind="ExternalOutput")

    with tile.TileContext(nc) as tc:
        with tc.tile_pool(name="singles", bufs=1) as singles:
            xh_s = singles.tile([_K, _N], bf16)
            yh_s = singles.tile([_K, _N], bf16)
            # separate queues so the two input DMAs overlap
            nc.sync.dma_start(out=xh_s[:, :], in_=xh_d.ap())
            nc.scalar.dma_start(out=yh_s[:, :], in_=yh_d.ap())

            # The PE produces NEGATED d2 (host negates the yh rows), so all
            # mins become maxes — letting direction A use the MAX8 unit.
            # Running max over m (partition direction), bf16, on DVE.
            # mt 0 initializes it with 4x-rate copies instead of TT+memset.
            accBb = singles.tile([128, _N], bf16)

            # per-mt MAX8 results (direction A): 8 values per mt, col 0 is
            # the max
            m8all = singles.tile([128, _MT * 8], bf16)

            # Each PSUM tile has exactly ONE consumer (the scalar engine's
            # bf16 cast) — multiple consumers of a PSUM tile are serialized
            # by the tile scheduler's bank tracker, which would gate the PE.
            # All max work then runs off the bf16 SBUF copy.
            with tc.tile_pool(name="psum", bufs=2, space="PSUM") as psum, \
                 tc.tile_pool(name="castp", bufs=4) as castp, \
                 tc.tile_pool(name="halfp", bufs=4) as halfp:
                for mt in range(_MT):
                    lhsT = yh_s[:, mt * 128:(mt + 1) * 128]
                    ptb = castp.tile([128, _N], bf16, name="ptb")
                    for c in range(_NCH):
                        pt = psum.tile([128, _CHUNK], f32)
                        for j in range(_CHUNK // 512):
                            n0 = c * _CHUNK + j * 512
                            nc.tensor.matmul(
                                pt[:, j * 512:(j + 1) * 512],
                                lhsT=lhsT,
                                rhs=xh_s[:, n0:n0 + 512],
                                start=True, stop=True,
                            )
                        if mt == 0 and c == 0:
                            # DVE is idle during the ramp: cast chunk 0
                            # itself (psum stays single-consumer; the
                            # scalar engine takes over from chunk 1 on)
                            nc.vector.tensor_copy(
                                ptb[:, 0:_CHUNK], pt[:, :])
                        else:
                            nc.scalar.copy(
                                out=ptb[:, c * _CHUNK:(c + 1) * _CHUNK],
                                in_=pt[:, :])
                        if mt == 0:
                            nc.vector.tensor_copy(
                                accBb[:, c * _CHUNK:(c + 1) * _CHUNK],
                                ptb[:, c * _CHUNK:(c + 1) * _CHUNK])
                    # direction B: elementwise running max (bf16 2x mode)
                    if mt > 0:
                        nc.vector.tensor_tensor(
                            out=accBb[:, :], in0=accBb[:, :],
                            in1=ptb[:, :], op=MAX)
                    # direction A: 3 pair-max halving levels (2x mode),
                    # then the MAX8 unit on the remaining 512 columns
                    h1 = halfp.tile([128, 2048], bf16, name="h1")
                    nc.vector.tensor_tensor(
                        out=h1[:, :], in0=ptb[:, 0:2048],
                        in1=ptb[:, 2048:4096], op=MAX)
                    h2 = halfp.tile([128, 1024], bf16, name="h2")
                    nc.vector.tensor_tensor(
                        out=h2[:, :], in0=h1[:, 0:1024],
                        in1=h1[:, 1024:2048], op=MAX)
                    h3 = halfp.tile([128, 512], bf16, name="h3")
                    nc.vector.tensor_tensor(
                        out=h3[:, :], in0=h2[:, 0:512],
                        in1=h2[:, 512:1024], op=MAX)
                    nc.vector.max(m8all[:, mt * 8:(mt + 1) * 8], h3[:, :])

            # ---- epilogue ----
            dirA = singles.tile([128, _MT], f32)
            nc.vector.tensor_reduce(
                out=dirA[:, :],
                in_=m8all[:, :].rearrange("p (m e) -> p m e", e=8),
                axis=X, op=MAX)

            identb = singles.tile([128, 128], bf16)
            make_identity(nc, identb[:, :])

            dirB = singles.tile([128, _MT], f32)
            with tc.tile_pool(name="tpsum", bufs=4, space="PSUM") as tpsum:
                for g in range(_N // 512):
                    tp = tpsum.tile([128, 512], bf16, name="tpb")
                    for i in range(4):
                        b = g * 4 + i
                        nc.tensor.transpose(
                            tp[:, i * 128:(i + 1) * 128],
                            accBb[:, b * 128:(b + 1) * 128],
                            identb[:, :])
                    # reduce only the innermost 128 (old partition axis);
                    # the 4 transposed blocks stay separate columns
                    nc.vector.tensor_reduce(
                        out=dirB[:, g * 4:(g + 1) * 4],
                        in_=tp[:, :].rearrange("p (a b) -> p a b", b=128),
                        axis=X, op=MAX)

            # dirA/dirB hold M = max(-d2) = -min(d2) for 2*4096 points; the
            # final clamp/sqrt/sum runs on the host (0.4% of the FLOPs),
            # which avoids the Sqrt act-table load and the serial tail.
            nc.sync.dma_start(out=out_d.ap()[:, 0:_MT], in_=dirA[:, :])
            nc.sync.dma_start(out=out_d.ap()[:, _MT:2 * _MT], in_=dirB[:, :])

    nc.compile()
    return nc


def _get_program():
    global _PROGRAM
    if _PROGRAM is None:
        _PROGRAM = _build_program()
    return _PROGRAM


def _split3(a):
    """fp32 array -> 3-level bf16 split (h1 + h2 + h3 ~ a to ~2^-26 rel)."""
    h1 = a.astype(_BF16)
    r1 = a - h1.astype(np.float32)
    h2 = r1.astype(_BF16)
    r2 = r1 - h2.astype(np.float32)
    h3 = r2.astype(_BF16)
    return h1, h2, h3


def _augment(x, y):
    """x, y: (4096, 3) f32 -> xh, yh (30, 4096) bf16 such that
    sum_k yh[k, m] * xh[k, n] == |y[m] - x[n]|^2 to ~1e-6 abs.

    Every fp32 operand is split into 3 bf16 levels; all product pairs down
    to the 2^-24 level are kept, so each product is exact in the PE's fp32
    PSUM accumulation.  Large-magnitude rows (y_sq, x_sq, hi*hi cross
    terms) come first so the running PSUM partial cancels down to ~d2
    early, keeping sequential-accumulation rounding at the fp32 noise
    floor of the reference itself."""
    xt = np.ascontiguousarray(x.T.astype(np.float32))            # (3, N)
    y2t = np.ascontiguousarray((-2.0 * y).T.astype(np.float32))  # (3, N)
    xsq = np.einsum("nd,nd->n", x, x).astype(np.float32)         # (N,)
    ysq = np.einsum("nd,nd->n", y, y).astype(np.float32)

    g1, g2, g3 = _split3(xt)
    h1, h2, h3 = _split3(y2t)
    xs1, xs2, xs3 = _split3(xsq)
    ys1, ys2, ys3 = _split3(ysq)
    ones = np.ones(_N, dtype=_BF16)

    xrows, yrows = [], []

    def add(xr, yr):
        xrows.append(xr)
        yrows.append(yr)

    add(ones, ys1)
    add(xs1, ones)
    for d in range(3):
        add(g1[d], h1[d])
    add(ones, ys2)
    add(ones, ys3)
    add(xs2, ones)
    add(xs3, ones)
    for d in range(3):
        add(g2[d], h1[d])
        add(g1[d], h2[d])
        add(g3[d], h1[d])
        add(g2[d], h2[d])
        add(g1[d], h3[d])
        add(g3[d], h2[d])
        add(g2[d], h3[d])
    xh = np.stack(xrows).astype(_BF16)
    # negate the y side so the PE emits -d2 (mins become maxes on-device)
    yh = (-np.stack(yrows).astype(np.float32)).astype(_BF16)
    assert xh.shape == (_K, _N)
    return xh, yh


def kernel(x1, y1):
    from concourse.bass_utils import run_bass_kernel_spmd

    x1 = np.asarray(x1)
    y1 = np.asarray(y1)
    assert x1.shape == (_B, _N, 3) and y1.shape == (_B, _N, 3)

    nc = _get_program()
    in_maps = []
    for b in range(_B):
        xh, yh = _augment(x1[b], y1[b])
        in_maps.append({"xh": xh, "yh": yh})

    res = run_bass_kernel_spmd(nc, in_maps, list(range(_NCORES)))
    total = 0.0
    for c in range(_NCORES):
        m = res.results[c]["out"].astype(np.float32)  # (128, 64) = -d2min
        dist = np.sqrt(1.0e-8 + np.maximum(-m, 0.0), dtype=np.float32)
        total += float(dist.sum(dtype=np.float64))
    return np.float32(total / (_B * _N))
